# revision 4
# baseline (speedup 1.0000x reference)
"""Mixture-of-Experts Trainium2 kernel (8 NeuronCores, expert-parallel).

Problem (fixed shapes): B=8192 tokens, D=1024 in, H=512 hidden, O=1024 out,
E=8 experts, top-K=2 routing, eval mode (no gate noise).

Strategy
--------
Expert-parallel with host-side routing (the "all-to-all dispatch tokens by
top-k expert id" option of the sharding hint, with the host acting as the
dispatcher):

1. Host computes the tiny gating network (x @ gate_W: 0.1% of total FLOPs)
   with jax on CPU, exactly as the reference does, so top-k selection and the
   aux loss match the reference bit-for-bit.
2. Tokens are dispatched by top-k expert id: core c receives the (transposed,
   zero-padded) tokens routed to expert c plus expert c's weights, and runs
   the expert MLP  w * relu(x @ W1 + b1) @ W2  on device. The per-token
   combine weight w is folded into the device kernel (per-partition scale on
   the output tiles), so each core emits already-scaled contributions.
3. Host sums the K=2 scaled contributions per token (a gather + one add) and
   adds the (usually zero) b2 term analytically: sum_k g_k * b2[e_k].

Only K/E = 1/4 of the dense all-expert FLOPs are executed, which together
with bf16 matmuls puts the kernel near both the PE and HBM rooflines.

Device kernel layout (per core, expert e, T = padded token count):
  xt  [D, T]    x^T gathered for this expert         (bf16)
  w1  [D, H]    W1[e] — used directly as matmul lhsT (bf16)
  w2  [H, O]    W2[e] — used directly as matmul rhs  (bf16)
  b1  [128,H/128]  b1[e] laid out partition-major for the fused relu bias
  wg  [128,T/128]  combine weights, partition-major for the output scale
  y   [T, O]    w * (relu(x@W1+b1) @ W2)  (fp32)

Layer 1 computes h^T (hidden on partitions): lhsT = W1 block [dxh], moving =
x^T block [dxt] -> psum [h, t]; relu(+b1 per-partition bias) -> SBUF bf16.
Layer 2 computes y (tokens on partitions): lhsT = h^T block [hxt], moving =
W2 block [hxo] -> psum [t, o]; scale by wg (per-partition) -> SBUF -> HBM.
No transposes anywhere on device.
"""

import os
import sys

import numpy as np

for _p in ("/opt/trn_rl_repo", "/root/.axon_site/_ro/trn_rl_repo"):
    if os.path.isdir(_p) and _p not in sys.path:
        sys.path.insert(0, _p)
        break

B, D, O, E, TOPK, H = 8192, 1024, 1024, 8, 2, 512
LOAD_BALANCE_ALPHA = 0.01
NCORES = 8
P = 128  # SBUF partitions

# "bf16" (fast, ~0.3% rel err) or "f32r" (full fp32 operands, PE fp32r path)
DTYPE_MODE = os.environ.get("MOE_DTYPE", "bf16")


def _make_patched_tc():
    """TileContext subclass working around a walrus codegen limit in this
    toolchain: instructions accept at most ONE sync wait ("Too many sync wait
    commands" in setupSyncWait otherwise). Two changes:

    1. _commit_instruction: any instruction carrying N>1 waits is prefixed
       with N-1 single-wait EventSemaphore instructions on the same engine
       (the engine executes its queue in order, so blocking semantics are
       identical), leaving one wait on the instruction itself.
    2. _drain_and_barrier: the kernel-tail Drain (TPB_CTRL — supports no
       sync-wait struct at all) gets its waits emitted as a chain of
       single-wait EVSEMs on SP, then drains with no waits.
    """
    import concourse.tile as tile
    from concourse import mybir
    from concourse.tile import ScopedClock

    class PatchedTC(tile.TileContext):
        def _commit_instruction(self, inst, lazy_reg_writes: bool = True):
            si = inst.sync_info
            if si is not None and si.on_wait and len(si.on_wait) > 1:
                waits = list(si.on_wait)
                for w in waits[:-1]:
                    ev = mybir.InstEventSemaphore(
                        name=self.nc.get_next_instruction_name(),
                        ins=[],
                        outs=[],
                        engine=inst.engine,
                    )
                    ev.sync_info = mybir.SyncInfo(on_wait=[w], on_update=[])
                    super()._commit_instruction(ev, lazy_reg_writes=False)
                inst.sync_info = mybir.SyncInfo(
                    on_wait=[waits[-1]], on_update=list(si.on_update or [])
                )
            return super()._commit_instruction(inst, lazy_reg_writes=lazy_reg_writes)

        def _drain_and_barrier(self, tick_clock, wait_clock):
            nc = self.nc
            assert self.sems is not None
            by_name = {h.name: h for h in self.sems.allocated().values()}
            di = mybir.InstNoOp(
                name="dummy-waits", ins=[], outs=[], engine=mybir.EngineType.SP
            )
            wait_clock.add_sem_waits(di, ScopedClock({None: tick_clock.global_clock}))
            waits = list(di.sync_info.on_wait or []) if di.sync_info else []
            for w in waits:
                h = by_name.get(w.ant_name)
                assert h is not None, f"no sem handle for {w.ant_name}"
                assert w.wait_mode == "sem-ge-imm", w.wait_mode
                nc.sync.wait_ge(h, w.wait_value)
            nc.sync.drain()
            nc.all_engine_barrier()
            popped = nc._tile_sem_poison_stack.pop()
            assert popped is self._sem_poison
            nc.clear_and_free_semaphores(list(self.sems.allocated().values()))
            nc.all_engine_barrier()

    return PatchedTC


def _build_program(t_pad: int, dtype_mode: str):
    """Build the single-core Bass program (SPMD: all 8 cores run this)."""
    import concourse.bass as bass
    import concourse.tile as tile
    from concourse import mybir

    f32 = mybir.dt.float32
    dt_in = mybir.dt.bfloat16 if dtype_mode == "bf16" else mybir.dt.float32r

    nd, nh, no = D // P, H // P, O // 512  # 8, 4, 2
    n_tb = t_pad // P  # 128-token blocks
    chunks = []  # (t0, csz) token chunks of <=512
    t0 = 0
    while t0 < t_pad:
        csz = min(512, t_pad - t0)
        chunks.append((t0, csz))
        t0 += csz

    nc = bass.Bass()
    xt_d = nc.dram_tensor("xt", (D, t_pad), dt_in, kind="ExternalInput")
    w1_d = nc.dram_tensor("w1", (D, H), dt_in, kind="ExternalInput")
    w2_d = nc.dram_tensor("w2", (H, O), dt_in, kind="ExternalInput")
    b1_d = nc.dram_tensor("b1", (P, nh), f32, kind="ExternalInput")
    wg_d = nc.dram_tensor("wg", (P, n_tb), f32, kind="ExternalInput")
    y_d = nc.dram_tensor("y", (t_pad, O), f32, kind="ExternalOutput")

    relu = mybir.ActivationFunctionType.Relu

    PatchedTC = _make_patched_tc()
    with PatchedTC(nc) as tc:
        with (
            tc.tile_pool(name="const", bufs=1) as cpool,
            tc.tile_pool(name="xtp", bufs=1) as xpool,
            tc.tile_pool(name="htp", bufs=1) as hpool,
            tc.tile_pool(name="outp", bufs=4) as opool,
            tc.tile_pool(name="ps1", bufs=2, space="PSUM") as ps1pool,
            tc.tile_pool(name="ps2", bufs=4, space="PSUM") as ps2pool,
        ):
            w1_sb = [cpool.tile([P, H], dt_in, name=f"w1_{db}") for db in range(nd)]
            w2_sb = [cpool.tile([P, O], dt_in, name=f"w2_{hb}") for hb in range(nh)]
            b1_sb = cpool.tile([P, nh], f32, name="b1_sb")
            wg_sb = cpool.tile([P, n_tb], f32, name="wg_sb")
            xt_sb = [xpool.tile([P, t_pad], dt_in, name=f"xt_{db}") for db in range(nd)]
            ht_sb = [hpool.tile([P, t_pad], dt_in, name=f"ht_{hb}") for hb in range(nh)]

            for db in range(nd):
                nc.sync.dma_start(w1_sb[db][:], w1_d[db * P : (db + 1) * P, :])
            for hb in range(nh):
                nc.sync.dma_start(w2_sb[hb][:], w2_d[hb * P : (hb + 1) * P, :])
            nc.sync.dma_start(b1_sb[:], b1_d[:])
            nc.sync.dma_start(wg_sb[:], wg_d[:])

            for t0, csz in chunks:
                # stream this chunk's tokens
                for db in range(nd):
                    nc.sync.dma_start(
                        xt_sb[db][:, t0 : t0 + csz],
                        xt_d[db * P : (db + 1) * P, t0 : t0 + csz],
                    )
                # layer 1: h^T[hb, t0:t0+csz] = relu(W1.T x + b1)
                for hb in range(nh):
                    ps1 = ps1pool.tile([P, 512], f32, tag="ps1", name=f"ps1_{t0}_{hb}")
                    for db in range(nd):
                        nc.tensor.matmul(
                            ps1[:, :csz],
                            w1_sb[db][:, hb * P : (hb + 1) * P],
                            xt_sb[db][:, t0 : t0 + csz],
                            start=(db == 0),
                            stop=(db == nd - 1),
                        )
                    nc.scalar.activation(
                        ht_sb[hb][:, t0 : t0 + csz],
                        ps1[:, :csz],
                        relu,
                        bias=b1_sb[:, hb : hb + 1],
                    )
                # layer 2: y[t, :] = wg[t] * (h^T.T @ W2)
                for tb in range(t0 // P, (t0 + csz) // P):
                    for ob in range(no):
                        ps2 = ps2pool.tile(
                            [P, 512], f32, tag="ps2", name=f"ps2_{tb}_{ob}"
                        )
                        for hb in range(nh):
                            nc.tensor.matmul(
                                ps2[:],
                                ht_sb[hb][:, tb * P : (tb + 1) * P],
                                w2_sb[hb][:, ob * 512 : (ob + 1) * 512],
                                start=(hb == 0),
                                stop=(hb == nh - 1),
                            )
                        out_sb = opool.tile([P, 512], f32, tag="out", name=f"o_{tb}_{ob}")
                        nc.vector.tensor_scalar_mul(
                            out_sb[:], ps2[:], wg_sb[:, tb : tb + 1]
                        )
                        nc.sync.dma_start(
                            y_d[tb * P : (tb + 1) * P, ob * 512 : (ob + 1) * 512],
                            out_sb[:],
                        )
    return nc


def _gate_host(x, gate_W, gate_b):
    """Gating + aux loss on CPU jax, matching the reference bit-for-bit."""
    import jax
    import jax.numpy as jnp

    cpu = jax.devices("cpu")[0]
    with jax.default_device(cpu):
        xj = jax.device_put(np.asarray(x), cpu)
        wj = jax.device_put(np.asarray(gate_W), cpu)
        bj = jax.device_put(np.asarray(gate_b), cpu)
        gate_logits = xj @ wj + bj
        top_v, top_i = jax.lax.top_k(gate_logits, TOPK)
        top_gates = jax.nn.softmax(top_v, axis=-1)
        gates = jax.nn.softmax(gate_logits, axis=-1)
        importance = gates.mean(axis=0)
        loss = LOAD_BALANCE_ALPHA * (E * jnp.sum(importance**2))
    return (
        np.asarray(top_i),
        np.asarray(top_gates, dtype=np.float32),
        np.asarray(loss, dtype=np.float32),
    )


def kernel(x, gate_W, gate_b, W1, b1, W2, b2):
    from concourse.bass_utils import run_bass_kernel_spmd

    x = np.ascontiguousarray(np.asarray(x, dtype=np.float32))
    W1 = np.asarray(W1, dtype=np.float32)
    b1 = np.asarray(b1, dtype=np.float32)
    W2 = np.asarray(W2, dtype=np.float32)
    b2 = np.asarray(b2, dtype=np.float32)

    top_i, top_gates, loss = _gate_host(x, gate_W, gate_b)

    # --- dispatch: sort (token, k) pairs by expert ---
    pair_tok = np.repeat(np.arange(B, dtype=np.int64), TOPK)
    pair_exp = top_i.reshape(-1).astype(np.int64)
    pair_g = top_gates.reshape(-1)
    order = np.argsort(pair_exp, kind="stable")
    sorted_tok = pair_tok[order]
    sorted_g = pair_g[order]
    counts = np.bincount(pair_exp, minlength=E)
    offs = np.zeros(E + 1, dtype=np.int64)
    offs[1:] = np.cumsum(counts)

    t_pad = max(512, int(-(-counts.max() // P) * P))  # round up to 128, min 512

    if DTYPE_MODE == "bf16":
        import ml_dtypes

        np_in = ml_dtypes.bfloat16
    else:
        np_in = np.float32

    nc = _build_program(t_pad, DTYPE_MODE)

    nh, n_tb = H // P, t_pad // P
    in_maps = []
    for e in range(E):
        idx = sorted_tok[offs[e] : offs[e + 1]]
        cnt = len(idx)
        xt = np.zeros((D, t_pad), dtype=np_in)
        xt[:, :cnt] = x[idx].T.astype(np_in)
        wg = np.zeros(t_pad, dtype=np.float32)
        wg[:cnt] = sorted_g[offs[e] : offs[e + 1]]
        in_maps.append(
            {
                "xt": xt,
                "w1": np.ascontiguousarray(W1[e].astype(np_in)),
                "w2": np.ascontiguousarray(W2[e].astype(np_in)),
                "b1": np.ascontiguousarray(b1[e].reshape(nh, P).T),
                "wg": np.ascontiguousarray(wg.reshape(n_tb, P).T),
            }
        )

    res = run_bass_kernel_spmd(nc, in_maps, list(range(NCORES)))

    # --- combine: each token has exactly K=2 scaled contributions ---
    y_cat = np.concatenate(
        [res.results[e]["y"][: counts[e]] for e in range(E)], axis=0
    )
    ord2 = np.argsort(sorted_tok, kind="stable")
    y_tok = y_cat[ord2]
    out = y_tok[0::2] + y_tok[1::2]

    if np.any(b2):
        for k in range(TOPK):
            out += top_gates[:, k : k + 1] * b2[top_i[:, k]]

    return out.astype(np.float32), loss
